# revision 19
# baseline (speedup 1.0000x reference)
"""Trainium2 Bass kernel: GroupNorm + single-head self-attention + residual.

Reference computation (B=4, C=256, L=4096, GROUPS=8):
    xn   = GroupNorm(x) * gn_w + gn_b
    qkv  = w_qkv @ xn + b_qkv          # 1x1 conv
    attn = softmax(q^T k / sqrt(C))
    out  = w_out @ (attn @ v) + b_out + x

Sharding: 8 cores = (batch b, query-half h). Each core computes GN stats and
k/v over all L (redundant with its sibling core, but cheap), and q/attention/
output projection only for its 2048 query positions. No collectives.

fp8 design (vs the fp32r baseline, ~2.4x faster):
  - x is shipped twice: e4m3 (matmul operand + GN stats source; quantization
    shifts the group stats by O(1e-3) relative — measured harmless) and fp32
    (residual only, DMA'd late, off the critical path).
  - GN scale/shift are folded into the qkv weights on device; the folded
    weights are quantized to e4m3 and all qkv projections run as DoubleRow
    fp8 matmuls (2 c-tiles contracted per MM at 0.5 cyc/col).
  - q/k carry sqrt(scale) = 1/4 each (folded on host pre-quantization) so
    both land ~N(0, 1/16) in e4m3's normal range.
  - Scores are computed transposed (scoresT[j,i] = k.q) via DoubleRow with a
    k-tile stationary: softmax reduction over j stays a PE ones-matmul and
    attn feeds the attn@v DoubleRow directly; exp stores e^s/4 in e4m3 (max
    score 6.5 -> 116 < 240, no overflow; the /4 cancels in softmax).
  - exp is split across engines: ACT uses the exp table (bias -2ln2); DVE
    computes e4m3 BITS directly with one tensor_scalar (Schraudolph:
    bits = round(8*log2(e)*s + 8*(7-sigma) - 16), f32->u8 is RNE+saturate
    on HW so negatives clamp to 0 = fp8 zero). Both paths feed the same
    sums/av matmuls, so softmax stays exactly normalized.
  - attn@v contracts a 256-key pair per MM (v pair stationary); softmax sums
    use a 16-byte-strided ones-pair stationary (dual-fp8 LDW requires the
    pair stride % 16 == 0).
  - Output projection stays fp32r (quantizing attn@v would overflow e4m3 and
    costs ~3e-2 relative error); per-chunk normalization/projection/residual
    is deferred into the next chunk's j-loop as in the baseline.
"""

import numpy as np
import ml_dtypes

import concourse.bass as bass
import concourse.mybir as mybir
from concourse import bacc
import concourse.tile as tile
from concourse.bass_utils import run_bass_kernel_spmd

P = 128
C = 256
L = 4096
LH = 2048  # query positions per core
B = 4
N_CORES = 8
CT = C // P  # 2 c-tiles
JT = L // P  # 32 j-tiles
EPS = 1e-5

F32 = mybir.dt.float32
F32R = mybir.dt.float32r
F8 = mybir.dt.float8e4
U8 = mybir.dt.uint8
E4NP = ml_dtypes.float8_e4m3fn

AF = mybir.ActivationFunctionType
ALU = mybir.AluOpType
DR = mybir.MatmulPerfMode.DoubleRow

# Softmax scale 1/sqrt(C) is applied inside exp (ACT scale= / DVE slope), so
# q/k weights stay unscaled in e4m3's clean normal range. Raw scores ~N(0,256).
SM_SCALE = 1.0 / 16.0
# Schraudolph e4m3-bit exp: bits = round(EXP_A*s_raw + EXP_C); value ~= e^s/4.
EXP_A = 11.5415603 * SM_SCALE  # 8 * log2(e) / 16
EXP_C = 39.6435                # 8 * (7 - 0.0450) - 16
M2LN2 = -1.38629436            # ACT-path bias: exp(s/16 - 2ln2) = e^s / 4
AV_SCALE = 1.0 / 16.0   # av -> e4m3 prescale; undone in the res epilogue
WO_SCALE = 8.0          # host prescale of wo8 (keeps w_out out of subnormals)
RES_SCALE = 1.0 / (AV_SCALE * WO_SCALE)  # = 2.0

# which j-pairs (of 16 per chunk) DVE handles; the rest go to ACT
DVE_PAIRS = frozenset((1, 4, 7, 10, 13))


def _r(ap):
    return ap.bitcast(F32R)


def build_nc(compile: bool = True, reps: int = 1):
    nc = bacc.Bacc("TRN2")

    # --- I/O ----------------------------------------------------------------
    x8q_d = nc.declare_dram_parameter("x8q", [C, LH], F8, isOutput=False)
    x8o_d = nc.declare_dram_parameter("x8o", [C, LH], F8, isOutput=False)
    xq_d = nc.declare_dram_parameter("xq", [C, LH], F32, isOutput=False)
    wqkvT_d = nc.declare_dram_parameter("wqkvT", [C, 3 * C], F32, isOutput=False)
    bqkv_d = nc.declare_dram_parameter("bqkv6", [P, 6], F32, isOutput=False)
    woutT_d = nc.declare_dram_parameter("woutT", [C, C], F32, isOutput=False)
    bout_d = nc.declare_dram_parameter("bout2", [P, CT], F32, isOutput=False)
    gnw_d = nc.declare_dram_parameter("gnw2", [P, CT], F32, isOutput=False)
    gnb_d = nc.declare_dram_parameter("gnb2", [P, CT], F32, isOutput=False)
    sel_d = nc.declare_dram_parameter("sel", [P, 4], F32, isOutput=False)
    selT_d = nc.declare_dram_parameter("selT", [4, P], F32, isOutput=False)
    wo8_d = nc.declare_dram_parameter("wo8", [C, C], F8, isOutput=False)
    out_d = nc.declare_dram_parameter("out", [C, LH], F32, isOutput=True)

    from concourse.tile_rust import add_dep_helper

    with tile.TileContext(nc) as tc, \
         tc.tile_pool(name="const", bufs=1) as const, \
         tc.tile_pool(name="xbuf", bufs=1) as xbuf, \
         tc.tile_pool(name="qkv", bufs=1) as qkvp, \
         tc.tile_pool(name="work", bufs=3) as work, \
         tc.tile_pool(name="res", bufs=3) as resp, \
         tc.tile_pool(name="exppool", bufs=3) as exppool, \
         tc.tile_pool(name="ps_big", bufs=2, space="PSUM") as ps_big, \
         tc.tile_pool(name="ps_av", bufs=1, space="PSUM") as ps_av, \
         tc.tile_pool(name="ps_small", bufs=1, space="PSUM") as ps_small:

        def emit_body():
            # --- x8 loads (chunked so stats can start early) ---------------
            x8q = xbuf.tile([P, CT, LH], F8)
            x8o = xbuf.tile([P, CT, LH], F8)
            x8q3 = x8q_d[:].rearrange("(t p) l -> p t l", p=P)
            x8o3 = x8o_d[:].rearrange("(t p) l -> p t l", p=P)
            NCH = 4  # dma chunks per (tensor, c-tile)
            CW = LH // NCH
            for t in range(CT):
                for n in range(NCH):
                    sl = slice(n * CW, (n + 1) * CW)
                    nc.sync.dma_start(x8q[:, t, sl], x8q3[:, t, sl])
                    xo_eng = nc.gpsimd if n == NCH - 1 else nc.sync
                    xo_eng.dma_start(x8o[:, t, sl], x8o3[:, t, sl])
            # fp32 x for the residual only - off the stats critical path
            xq = xbuf.tile([P, CT, LH], F32)
            xq3 = xq_d[:].rearrange("(t p) l -> p t l", p=P)
            for t in range(CT):
                for n in range(NCH):
                    sl = slice(n * CW, (n + 1) * CW)
                    nc.gpsimd.dma_start(xq[:, t, sl], xq3[:, t, sl])

            # Preload the exp ACT table set while x streams in. warm == exp(0)
            # == 1.0, multiplied into the group rstd below to survive DCE.
            warm = work.tile([4, 1], F32, tag="warm")
            nc.vector.memset(warm, 0.0)
            nc.scalar.activation(warm, warm, AF.Exp)

            # --- constant / weight loads -----------------------------------
            wT = const.tile([P, CT, 3 * C], F32)   # wqkvT[c_in, c_out] tiled
            nc.sync.dma_start(wT, wqkvT_d[:].rearrange("(t p) o -> p t o", p=P))
            woT = const.tile([P, CT, C], F32R)
            nc.gpsimd.dma_start(woT, _r(woutT_d[:].rearrange("(t p) o -> p t o", p=P)))
            bqkv = const.tile([P, 6], F32)
            nc.sync.dma_start(bqkv, bqkv_d[:])
            bout = const.tile([P, CT], F32)
            nc.sync.dma_start(bout, bout_d[:])
            gnw = const.tile([P, CT], F32)
            nc.sync.dma_start(gnw, gnw_d[:])
            gnb = const.tile([P, CT], F32)
            nc.sync.dma_start(gnb, gnb_d[:])
            sel = const.tile([P, 4], F32R)
            nc.gpsimd.dma_start(sel, _r(sel_d[:]))
            selT = const.tile([4, P], F32R)
            nc.gpsimd.dma_start(selT, _r(selT_d[:]))
            wo8 = const.tile([P, CT, C], F8)
            nc.gpsimd.dma_start(wo8, wo8_d[:].rearrange("(t p) o -> p t o", p=P))
            # fp8 ones pair for DoubleRow sums (pair stride must be %16)
            ones8 = const.tile([P, 2, 16], F8)
            nc.vector.memset(ones8.bitcast(U8), 0x38)  # 1.0 in e4m3
            m2ln2 = const.tile([P, 1], F32)
            nc.vector.memset(m2ln2, M2LN2)

            # --- GroupNorm stats (from e4m3 x) -----------------------------
            # Per-channel mean/E[x^2], split across engines: bn_stats on DVE
            # for x8q + the first x8o chunk, Identity/Square+accum_out on the
            # otherwise-idle ACT engine for the remaining x8o chunks.
            SW = 512
            nst = LH // SW   # chunks per (half, c-tile)
            NDVE_XO = 2      # x8o chunks on DVE bn_stats (engine balance)
            NACT = nst - NDVE_XO  # x8o chunks handled by ACT per c-tile
            stats = work.tile([P, CT, nst + NDVE_XO, 6], F32, tag="bnstats")
            s_acc = work.tile([P, CT, NACT, 2], F32, tag="sacc")
            for t in range(CT):
                for n in range(nst):
                    sl = slice(n * SW, (n + 1) * SW)
                    nc.vector.bn_stats(stats[:, t, n, :], x8q[:, t, sl])
                for n in range(NDVE_XO):
                    sl = slice(n * SW, (n + 1) * SW)
                    nc.vector.bn_stats(stats[:, t, nst + n, :], x8o[:, t, sl])
                for i in range(NACT):
                    sl = slice((i + NDVE_XO) * SW, (i + NDVE_XO + 1) * SW)
                    scr = work.tile([P, SW], F32, tag="actscr")
                    nc.scalar.activation(scr, x8o[:, t, sl], AF.Identity,
                                         accum_out=s_acc[:, t, i, 0:1])
                    scr2 = work.tile([P, SW], F32, tag="actscr2")
                    nc.scalar.activation(scr2, x8o[:, t, sl], AF.Square,
                                         accum_out=s_acc[:, t, i, 1:2])
            # HAM warm-up: throwaway DoubleRow matmuls while the PE is idle
            # during stats, so the clock gate is at 8/8 when qkv starts.
            ps_w = ps_av.tile([4, 512], F32, tag="av")
            for wi in range(16):
                nc.tensor.matmul(ps_w, ones8[:, :, 0:4],
                                 x8q[:, :, (wi % 4) * 512:(wi % 4 + 1) * 512],
                                 start=(wi == 0), stop=(wi == 15), perf_mode=DR)
            wsum = work.tile([4, 1], F32, tag="wsum")
            nc.vector.tensor_copy(wsum, ps_w[:, 0:1])
            wone = work.tile([4, 1], F32, tag="wone")
            nc.vector.tensor_scalar(wone, wsum, 0.0, 1.0, ALU.mult, ALU.add)

            mv = work.tile([P, CT, 2], F32, tag="mv")  # bn-side (mean, var)
            for t in range(CT):
                nc.vector.bn_aggr(mv[:, t, :], stats[:, t, :, :])

            # Combine: mean = w_bn*m_bn + sum(s1)/N ; E2 = w_bn*(v+m^2) + sum(s2)/N
            W_BN = float(nst + NDVE_XO) / (2 * nst)
            INV_N = 1.0 / (2 * nst * SW)
            ssum = work.tile([P, CT, 2], F32, tag="ssum")
            nc.vector.reduce_sum(ssum, s_acc.rearrange("p t n k -> p t k n"),
                                 axis=mybir.AxisListType.X)
            rs = work.tile([P, CT, 2], F32R, tag="rs")
            # mean
            nc.vector.tensor_scalar(rs[:, :, 0], mv[:, :, 0], W_BN, None, ALU.mult)
            nc.vector.tensor_scalar(ssum[:, :, 0], ssum[:, :, 0], INV_N, None,
                                    ALU.mult)
            nc.vector.tensor_tensor(rs[:, :, 0], rs[:, :, 0].bitcast(F32),
                                    ssum[:, :, 0], ALU.add)
            # E[x^2]
            e2bn = work.tile([P, CT], F32, tag="e2bn")
            nc.vector.tensor_tensor(e2bn, mv[:, :, 0], mv[:, :, 0], ALU.mult)
            nc.vector.tensor_tensor(e2bn, e2bn, mv[:, :, 1], ALU.add)
            nc.vector.tensor_scalar(e2bn, e2bn, W_BN, None, ALU.mult)
            nc.vector.tensor_scalar(ssum[:, :, 1], ssum[:, :, 1], INV_N, None,
                                    ALU.mult)
            nc.vector.tensor_tensor(rs[:, :, 1], e2bn, ssum[:, :, 1], ALU.add)

            # group sums over the 32-channel groups: out[j, col] (j = p//32)
            ps_g = ps_small.tile([4, 2 * CT], F32, tag="bmat")
            nc.tensor.matmul(ps_g, sel.bitcast(F32),
                             rs.rearrange("p t k -> p (t k)").bitcast(F32),
                             start=True, stop=True)
            g_sb = work.tile([4, CT, 2], F32, tag="gsb")
            nc.vector.tensor_scalar_mul(g_sb, ps_g.rearrange("j (t k) -> j t k", k=2),
                                        1.0 / 32.0)
            # pk[:, 0:CT] = rstd_g per tile, pk[:, CT:2CT] = mean_g per tile
            pk = work.tile([4, 2 * CT], F32R, tag="pk")
            pk3 = pk.rearrange("j (a t) -> j a t", a=2)
            nc.vector.tensor_copy(pk3[:, 1, :], g_sb[:, :, 0])  # group means
            # var = E[x^2] - mean^2 ; rstd via Newton iteration from y0=1.5-v/2
            vg = work.tile([4, CT], F32, tag="vg")
            nc.vector.tensor_tensor(vg, g_sb[:, :, 0], g_sb[:, :, 0], ALU.mult)
            nc.vector.tensor_tensor(vg, g_sb[:, :, 1], vg, ALU.subtract)
            nc.vector.tensor_scalar_add(vg, vg, EPS)
            nwy = work.tile([4, CT], F32, tag="nwy")
            nc.vector.tensor_scalar(nwy, vg, -0.5, 1.5, ALU.mult, ALU.add)
            nwt = work.tile([4, CT], F32, tag="nwt")
            for _ in range(1):  # group var is within ~1% of 1; 1 step suffices
                nc.vector.tensor_tensor(nwt, nwy, nwy, ALU.mult)
                nc.vector.tensor_tensor(nwt, vg, nwt, ALU.mult)
                nc.vector.tensor_scalar(nwt, nwt, -0.5, 1.5, ALU.mult, ALU.add)
                nc.vector.tensor_tensor(nwy, nwy, nwt, ALU.mult)
            nc.vector.tensor_scalar_mul(pk3[:, 0, :], nwy, warm[:, 0:1])
            nc.vector.tensor_scalar_mul(pk3[:, 0, :], pk3[:, 0, :].bitcast(F32),
                                        wone)

            # broadcast group values to all 128 partitions via matmul with selT
            ps_bc = ps_small.tile([P, 2 * CT], F32, tag="bmat")
            nc.tensor.matmul(ps_bc, selT.bitcast(F32), pk.bitcast(F32),
                             start=True, stop=True)
            gb3 = ps_bc.rearrange("p (a t) -> p a t", a=2)

            # scale_c = rstd * gn_w ; shift_c = gn_b - mean * scale_c
            scale_c = work.tile([P, CT], F32, tag="scale_c")
            nc.vector.tensor_tensor(scale_c, gb3[:, 0, :], gnw, ALU.mult)

            # folded qkv weights, quantized to e4m3:  W'8 = e4m3(wT * scale_c)
            wp8 = const.tile([P, CT, 3 * C], F8)
            for t in range(CT):
                nc.vector.tensor_scalar_mul(wp8[:, t, :], wT[:, t, :],
                                            scale_c[:, t : t + 1])

            shift_c = work.tile([P, CT], F32R, tag="shift_c")
            nc.vector.tensor_tensor(shift_c, gb3[:, 1, :], scale_c, ALU.mult)
            nc.vector.tensor_tensor(shift_c, gnb, shift_c, ALU.subtract)

            # effective qkv bias: bias_eff = b_qkv + W @ shift  (fp32 exact)
            bias_eff = const.tile([P, 6], F32R)
            for mt in range(6):
                ps_b = ps_small.tile([P, 1], F32, tag="bmat")
                for t in range(CT):
                    nc.tensor.matmul(ps_b,
                                     wT[:, t, mt * P : (mt + 1) * P],
                                     shift_c[:, t : t + 1].bitcast(F32),
                                     start=(t == 0), stop=(t == CT - 1))
                nc.vector.tensor_tensor(bias_eff[:, mt : mt + 1], ps_b,
                                        bqkv[:, mt : mt + 1], ALU.add)

            # effective output bias: bout_eff = b_out + w_out @ bias_v
            bout_eff = const.tile([P, CT], F32)
            for mt in range(CT):
                ps_b = ps_small.tile([P, 1], F32, tag="bmat")
                for t in range(CT):
                    nc.tensor.matmul(ps_b,
                                     woT[:, t, mt * P : (mt + 1) * P].bitcast(F32),
                                     bias_eff[:, 4 + t : 5 + t].bitcast(F32),
                                     start=(t == 0), stop=(t == CT - 1))
                nc.vector.tensor_tensor(bout_eff[:, mt : mt + 1], ps_b,
                                        bout[:, mt : mt + 1], ALU.add)

            # --- q, k, v projections (DoubleRow fp8) -----------------------
            # q8[p, t, i] = q[t*128+p, i] for own half (carries 1/4 from host)
            q8 = qkvp.tile([P, CT, LH], F8)
            for mt in range(CT):
                for n in range(LH // 512):
                    sl = slice(n * 512, (n + 1) * 512)
                    ps_q = ps_big.tile([P, 512], F32, tag="big")
                    nc.tensor.matmul(ps_q, wp8[:, :, mt * P : (mt + 1) * P],
                                     x8q[:, :, sl], start=True, stop=True,
                                     perf_mode=DR)
                    if n % 2 == 0:
                        nc.scalar.activation(q8[:, mt, sl], ps_q, AF.Identity,
                                             bias=bias_eff[:, mt : mt + 1].bitcast(F32))
                    else:
                        nc.vector.tensor_scalar(q8[:, mt, sl], ps_q,
                                                bias_eff[:, mt : mt + 1].bitcast(F32),
                                                None, ALU.add)

            # k8: [c_out, j] over both halves (own half first = j order)
            k8 = qkvp.tile([P, CT, L], F8)
            for mt in range(CT):
                for h, xsrc in enumerate((x8q, x8o)):
                    for n in range(LH // 512):
                        sl = slice(n * 512, (n + 1) * 512)
                        osl = slice(h * LH + n * 512, h * LH + (n + 1) * 512)
                        ps_k = ps_big.tile([P, 512], F32, tag="big")
                        nc.tensor.matmul(
                            ps_k, wp8[:, :, (2 + mt) * P : (3 + mt) * P],
                            xsrc[:, :, sl], start=True, stop=True, perf_mode=DR)
                        if n % 2 == 0:
                            nc.scalar.activation(k8[:, mt, osl], ps_k, AF.Identity,
                                                 bias=bias_eff[:, 2 + mt : 3 + mt].bitcast(F32))
                        else:
                            nc.vector.tensor_scalar(k8[:, mt, osl], ps_k,
                                                    bias_eff[:, 2 + mt : 3 + mt].bitcast(F32),
                                                    None, ALU.add)

            # v8 transposed: [j, c] (no bias; folded into bout_eff)
            v8 = qkvp.tile([P, JT, C], F8)
            last_v_copy = None
            for jb in range(JT):
                xsrc = x8q if jb < JT // 2 else x8o
                off = (jb % (JT // 2)) * P
                ps_v = ps_big.tile([P, C], F32, tag="big")
                nc.tensor.matmul(ps_v, xsrc[:, :, off : off + P],
                                 wp8[:, :, 2 * C : 3 * C], start=True,
                                 stop=True, perf_mode=DR)
                if jb % 2 == 0:
                    last_v_copy = nc.scalar.activation(v8[:, jb, :], ps_v,
                                                       AF.Copy, bias=0.0)
                else:
                    last_v_copy = nc.vector.tensor_copy(v8[:, jb, :], ps_v)

            # --- attention -------------------------------------------------
            IC = 512  # query-chunk width
            out3 = out_d[:].rearrange("(t p) l -> p t l", p=P)

            def finish_phase1(av, sums):
                """ACT/DVE/GPS-only epilogue start: free the sums slot (recip
                first - the next chunk's sums matmul waits on it), release the
                av PSUM slot via prescaled e4m3 copies (split ACT/DVE), and
                broadcast 1/Z. No PE instructions, so it is emitted at the
                producing chunk's end without wedging the PE FIFO."""
                rec_f = work.tile([1, IC], F32, tag="recf")
                nc.vector.reciprocal(rec_f, sums)
                av8 = work.tile([P, CT, IC], F8, tag="av8")
                nc.scalar.activation(av8[:, 0, :], av[:, 0, :], AF.Copy,
                                     bias=0.0, scale=AV_SCALE)
                nc.vector.tensor_scalar_mul(av8[:, 1, :], av[:, 1, :],
                                            AV_SCALE)
                B_sb = work.tile([P, IC], F32, tag="bsb")
                nc.gpsimd.partition_broadcast(B_sb, rec_f)
                return av8, B_sb

            def finish_phase2(ch, av8, B_sb, splits=1):
                """fp8 output projection + normalize + bias + residual + DMA.
                Emitted a few pairs later so the PE proj matmuls queue after
                av8 is ready. res = (ps_o * RES_SCALE) * (1/Z) + bout + x."""
                SPW = IC // splits
                for sp in range(splits):
                    ssl = slice(sp * SPW, (sp + 1) * SPW)
                    isl = slice(ch * IC + sp * SPW, ch * IC + (sp + 1) * SPW)
                    ps_o = ps_big.tile([P, CT, IC], F32, tag="big")
                    for mt in range(CT):
                        nc.tensor.matmul(ps_o[:, mt, ssl],
                                         wo8[:, :, mt * P : (mt + 1) * P],
                                         av8[:, :, ssl], start=True, stop=True,
                                         perf_mode=DR)
                    res = resp.tile([P, CT, IC], F32, tag="res")
                    for mt in range(CT):
                        nc.vector.scalar_tensor_tensor(
                            res[:, mt, ssl], ps_o[:, mt, ssl], RES_SCALE,
                            B_sb[:, ssl], ALU.mult, ALU.mult)
                        nc.vector.scalar_tensor_tensor(
                            res[:, mt, ssl], res[:, mt, ssl],
                            bout_eff[:, mt : mt + 1], xq[:, mt, isl],
                            ALU.add, ALU.add)
                    nc.sync.dma_start(out3[:, :, isl], res[:, :, ssl])

            first_scores_mm = None
            pending = None
            for ch in range(LH // IC):
                isl = slice(ch * IC, (ch + 1) * IC)
                av = ps_av.tile([P, CT, IC], F32, tag="av")
                sums = ps_small.tile([1, IC], F32, tag="sums")

                def emit_av(jp, ex):
                    for ct in range(CT):
                        nc.tensor.matmul(av[:, ct, :],
                                         v8[:, 2 * jp : 2 * jp + 2,
                                            ct * P : (ct + 1) * P],
                                         ex,
                                         start=(jp == 0), stop=(jp == JT // 2 - 1),
                                         perf_mode=DR)
                    nc.tensor.matmul(sums, ones8[:, :, 0:1], ex,
                                     start=(jp == 0), stop=(jp == JT // 2 - 1),
                                     perf_mode=DR)

                # attn@v runs one j-pair behind the scores/exp pipeline so the
                # exp latency hides under the next pair's scores matmuls.
                prev = None
                for jp in range(JT // 2):
                    ps_s = ps_big.tile([P, 2, IC], F32, tag="big")
                    for jj in range(2):
                        j = 2 * jp + jj
                        mm = nc.tensor.matmul(
                            ps_s[:, jj, :],
                            k8[:, :, j * P : (j + 1) * P],
                            q8[:, :, isl],
                            start=True, stop=True, perf_mode=DR)
                        if first_scores_mm is None:
                            first_scores_mm = mm
                            add_dep_helper(mm.ins, last_v_copy.ins, True,
                                           "observe v8 before attention")
                    ex = exppool.tile([P, 2, IC], F8, tag="exp")
                    if jp in DVE_PAIRS:
                        nc.vector.tensor_scalar(ex.bitcast(U8), ps_s, EXP_A,
                                                EXP_C, ALU.mult, ALU.add)
                    else:
                        nc.scalar.activation(ex, ps_s, AF.Exp, bias=m2ln2,
                                             scale=SM_SCALE)
                    if jp == 3 and pending is not None:
                        finish_phase2(pending[0], *pend_p1)
                        pending = None
                    if prev is not None:
                        emit_av(*prev)
                    prev = (jp, ex)
                emit_av(*prev)
                pending = (ch, av, sums)
                pend_p1 = finish_phase1(av, sums)
            finish_phase2(pending[0], *pend_p1, splits=2)

        if reps > 1:
            with tc.For_i(0, reps, 1):
                emit_body()
        else:
            emit_body()

    if compile:
        nc.compile()
    return nc


def make_host_inputs(x, gn_w, gn_b, w_qkv, b_qkv, w_out, b_out):
    """Shared (weight) arrays + per-core (x8q, x8o, xq) shards."""
    wqkvT = np.ascontiguousarray(w_qkv.T).astype(np.float32)
    bqkv6 = np.ascontiguousarray(b_qkv.astype(np.float32).reshape(6, P).T)
    woutT = np.ascontiguousarray(w_out.T).astype(np.float32)
    wo8 = (woutT * WO_SCALE).astype(E4NP)
    bout2 = np.ascontiguousarray(b_out.astype(np.float32).reshape(CT, P).T)
    gnw2 = np.ascontiguousarray(gn_w.astype(np.float32).reshape(CT, P).T)
    gnb2 = np.ascontiguousarray(gn_b.astype(np.float32).reshape(CT, P).T)
    pidx = np.arange(P)
    sel = (pidx[:, None] // 32 == np.arange(4)[None, :]).astype(np.float32)
    selT = np.ascontiguousarray(sel.T)

    shared = dict(wqkvT=wqkvT, bqkv6=bqkv6, woutT=woutT, wo8=wo8, bout2=bout2,
                  gnw2=gnw2, gnb2=gnb2, sel=sel, selT=selT)

    in_maps = []
    for core in range(N_CORES):
        b, h = divmod(core, 2)
        own = slice(h * LH, (h + 1) * LH)
        oth = slice((1 - h) * LH, (2 - h) * LH)
        m = dict(shared)
        xq_f32 = np.ascontiguousarray(x[b][:, own]).astype(np.float32)
        xo_f32 = np.ascontiguousarray(x[b][:, oth]).astype(np.float32)
        m["xq"] = xq_f32
        m["x8q"] = xq_f32.astype(E4NP)
        m["x8o"] = xo_f32.astype(E4NP)
        in_maps.append(m)
    return in_maps


_NC = None


def kernel(x, gn_w, gn_b, w_qkv, b_qkv, w_out, b_out, _trace=False, **_kw):
    global _NC
    x = np.asarray(x)
    if _NC is None:
        _NC = build_nc()
    in_maps = make_host_inputs(np.asarray(x), np.asarray(gn_w), np.asarray(gn_b),
                               np.asarray(w_qkv), np.asarray(b_qkv),
                               np.asarray(w_out), np.asarray(b_out))
    kw = {}
    if _trace:
        kw = dict(trace=True)
    br = run_bass_kernel_spmd(_NC, in_maps, list(range(N_CORES)), **kw)
    out = np.empty((B, C, L), np.float32)
    for core in range(N_CORES):
        b, h = divmod(core, 2)
        out[b][:, h * LH : (h + 1) * LH] = br.results[core]["out"]
    if _trace:
        return out, br
    return out
